# revision 36
# baseline (speedup 1.0000x reference)
"""Cumulative LayerNorm (cLN) Trainium2 Bass kernel.

x: [B=8, C=512, T=16000] fp32.  Per (b, t):
    mean[t] = cumsum_t(sum_c x) / (C*(t+1))
    var[t]  = cumsum_t(sum_c (x - mean[t'])^2) / (C*(t+1))
    out     = (x - mean) / sqrt(var + eps) * gamma + beta

Expansion used on-device (exact in real arithmetic):
    sum_c (x[c,t'] - mean[t'])^2 = ssq[t'] - 2*mean[t']*s1[t'] + C*mean[t']^2

Sharding: data-parallel over batch, one batch per NeuronCore (8 cores).

Per-core pipeline, T processed in 5 chunks of 3200 so x is read from HBM only
once (the chunk stays resident in SBUF between the stats pass and the
normalization pass):
  Stats:   reduce over C via PE matmuls with an all-ones [128,1] stationary
           operand into PSUM rows s1/ssq [1,400];
           squares on ACT; rows evacuated to SBUF and DMA-reshaped into the
           chunk's compact scan layout [128p, 25f] (t_local = p*25 + f).
  Scan:    per-partition prefix sums via DVE tensor_tensor_scan; cross-
           partition carry via a strict-lower-triangular PE matmul; cross-
           chunk carry via a PSUM-accumulated grand total (g) broadcast with a
           second accumulating matmul; pointwise stats; inv = 1/sqrt(var+eps)
           (ACT sqrt + DVE reciprocal); nminv = -mean*inv.
  Norm:    inv/nminv reshaped to [1, 1600] rows (SBUF->SBUF DMA) and
           replicated across all 128 partitions by GPSIMD partition_broadcast
           (no HBM traffic); normalization runs fully in place in the x tiles
           (DVE mul + add, then one ACT affine folding gamma/beta); DMA out.

The ssq reduction matmuls use float32r (full-rate fp32): their input is the
ACT square with a float32r-rounded output, which the BIR verifier requires.
The s1 matmuls consume raw DMA-loaded x and must stay plain fp32 (4 cyc/row).

Built with Bacc (not raw Bass): its compile() pass legalizes multi-wait
instructions into event-semaphore chains — TRN2 hardware instructions can
carry only ONE sync wait.
"""

import numpy as np

B, C, T = 8, 512, 16000
P = 128
NCH = C // P        # 4 chunks of channels
CC = 3200           # t-chunk (must be P * F2 and divide T)
NCC = T // CC       # 5
F2 = CC // P        # 25: compact scan layout free dim per chunk
KB = 400            # PSUM-row block (<=512 fp32, 400 = 16*25)
NKB = CC // KB      # 8 blocks per chunk
HB = 1600           # normalization half-chunk
EPS = 1e-8

_PROG = None


def _build_program():
    from contextlib import ExitStack

    import concourse.bass as bass
    import concourse.tile as tile
    from concourse import bacc, mybir

    f32 = mybir.dt.float32
    f32r = mybir.dt.float32r
    Alu = mybir.AluOpType
    Act = mybir.ActivationFunctionType

    nc = bacc.Bacc("TRN2", debug=False)
    x = nc.dram_tensor("x", [C, T], f32, kind="ExternalInput").ap()
    lstrict = nc.dram_tensor("lstrict", [P, P], f32, kind="ExternalInput").ap()
    recip5 = nc.dram_tensor("recip5", [P, NCC, F2], f32, kind="ExternalInput").ap()
    gamma_pc = nc.dram_tensor("gamma_pc", [P, NCH], f32, kind="ExternalInput").ap()
    beta_pc = nc.dram_tensor("beta_pc", [P, NCH], f32, kind="ExternalInput").ap()
    out = nc.dram_tensor("out", [C, T], f32, kind="ExternalOutput").ap()

    with tile.TileContext(nc) as tc:
        with ExitStack() as ctx:
            singles = ctx.enter_context(tc.tile_pool(name="singles", bufs=1))
            xtp = ctx.enter_context(tc.tile_pool(name="xtp", bufs=11))
            sqp_pool = ctx.enter_context(tc.tile_pool(name="sqp_pool", bufs=4))
            bcp = ctx.enter_context(tc.tile_pool(name="bcp", bufs=3))
            rowp = ctx.enter_context(tc.tile_pool(name="rowp", bufs=4))
            statp = ctx.enter_context(tc.tile_pool(name="statp", bufs=2))
            ps_stat = ctx.enter_context(
                tc.tile_pool(name="ps_stat", bufs=6, space="PSUM")
            )
            ps_carry = ctx.enter_context(
                tc.tile_pool(name="ps_carry", bufs=1, space="PSUM")
            )
            ps_g = ctx.enter_context(tc.tile_pool(name="ps_g", bufs=1, space="PSUM"))

            # ---- constants ----
            ones_col = singles.tile([P, 1], f32)
            nc.vector.memset(ones_col, 1.0)
            ones_row = singles.tile([1, P], f32)
            nc.vector.memset(ones_row, 1.0)
            ones_scan = singles.tile([P, F2], f32)
            nc.vector.memset(ones_scan, 1.0)
            lstrict_sb = singles.tile([P, P], f32)
            nc.sync.dma_start(lstrict_sb, lstrict)
            recip_sb = singles.tile([P, NCC, F2], f32)
            nc.sync.dma_start(recip_sb, recip5)
            gamma_sb = singles.tile([P, NCH], f32)
            nc.sync.dma_start(gamma_sb, gamma_pc)
            beta_sb = singles.tile([P, NCH], f32)
            nc.sync.dma_start(beta_sb, beta_pc)
            eps_sb = singles.tile([P, 1], f32)
            nc.vector.memset(eps_sb, EPS)

            # grand totals over processed chunks: col 0 = sum(s1), col 1 = sum(r)
            g_ps = ps_g.tile([1, 2], f32, tag="g")

            for cc in range(NCC):
                t0 = cc * CC
                # ---- load chunk ----
                xts = []
                for j in range(NCH):
                    xtr = xtp.tile([P, CC], f32r, tag="xt", name=f"xt_{cc}_{j}")
                    nc.sync.dma_start(
                        xtr.bitcast(f32), x[j * P : (j + 1) * P, t0 : t0 + CC]
                    )
                    xts.append(xtr.bitcast(f32))

                # ---- stats: channel reductions ----
                s1c = statp.tile([P, F2], f32, tag="s1c", name=f"s1c_{cc}")
                sqc = statp.tile([P, F2], f32, tag="sqc", name=f"sqc_{cc}")
                for kp in range(NKB // 2):
                    xsqs = []
                    for j in range(NCH):
                        xsq = sqp_pool.tile(
                            [P, 2 * KB], f32r, tag="xsq", name=f"xsq_{cc}_{kp}_{j}"
                        )
                        nc.scalar.square(
                            xsq, xts[j][:, kp * 2 * KB : (kp + 1) * 2 * KB]
                        )
                        xsqs.append(xsq)
                    for k2 in range(2):
                        k = kp * 2 + k2
                        s1p = ps_stat.tile([1, KB], f32, tag="st", name=f"s1p_{cc}_{k}")
                        sqp = ps_stat.tile([1, KB], f32, tag="st", name=f"sqp_{cc}_{k}")
                        for j in range(NCH):
                            nc.tensor.matmul(
                                s1p,
                                ones_col,
                                xts[j][:, k * KB : (k + 1) * KB],
                                start=(j == 0),
                                stop=(j == NCH - 1),
                            )
                        for j in range(NCH):
                            nc.tensor.matmul(
                                sqp,
                                ones_col.bitcast(f32r),
                                xsqs[j][:, k2 * KB : (k2 + 1) * KB],
                                start=(j == 0),
                                stop=(j == NCH - 1),
                            )
                        s1row = rowp.tile(
                            [1, KB], f32, tag="rows", name=f"s1r_{cc}_{k}"
                        )
                        nc.vector.tensor_copy(s1row, s1p)
                        sqrow = rowp.tile(
                            [1, KB], f32, tag="rows", name=f"sqr_{cc}_{k}"
                        )
                        nc.scalar.copy(sqrow, sqp)
                        # 400 t's = 16 partitions x 25 in the chunk scan layout
                        nc.sync.dma_start(s1c[16 * k : 16 * k + 16, :], s1row)
                        nc.sync.dma_start(sqc[16 * k : 16 * k + 16, :], sqrow)

                # ---- scan + pointwise stats (compact [128, 25]) ----
                if cc > 0:
                    g_prev = statp.tile([1, 2], f32, tag="gprev", name=f"gp_{cc}")
                    nc.vector.tensor_copy(g_prev, g_ps)
                cum1 = statp.tile([P, F2], f32, tag="cum1", name=f"cum1_{cc}")
                nc.vector.tensor_tensor_scan(
                    cum1, ones_scan, s1c, 0.0, Alu.mult, Alu.add
                )
                carryb = ps_carry.tile([P, 2], f32, tag="c", name=f"c_{cc}")
                carry1 = carryb[:, 0:1]
                nc.tensor.matmul(
                    carry1,
                    lstrict_sb,
                    cum1[:, F2 - 1 : F2],
                    start=True,
                    stop=(cc == 0),
                )
                if cc > 0:
                    nc.tensor.matmul(
                        carry1,
                        ones_row,
                        g_prev[:, 0:1],
                        start=False,
                        stop=True,
                        skip_group_check=True,
                    )
                nc.tensor.matmul(
                    g_ps[:, 0:1],
                    ones_col,
                    cum1[:, F2 - 1 : F2],
                    start=(cc == 0),
                    stop=(cc == NCC - 1),
                    skip_group_check=True,
                )
                carry1_sb = statp.tile([P, 1], f32, tag="cs1", name=f"cs1_{cc}")
                nc.vector.tensor_copy(carry1_sb, carry1)
                rc = recip_sb[:, cc, :]
                mean_c = statp.tile([P, F2], f32, tag="mean", name=f"mean_{cc}")
                nc.vector.scalar_tensor_tensor(
                    mean_c, cum1, carry1_sb, rc, Alu.add, Alu.mult
                )
                u_c = statp.tile([P, F2], f32, tag="u", name=f"u_{cc}")
                nc.vector.scalar_tensor_tensor(
                    u_c, mean_c, -float(C) / 2.0, s1c, Alu.mult, Alu.add
                )
                v_c = statp.tile([P, F2], f32, tag="v", name=f"v_{cc}")
                nc.vector.tensor_mul(v_c, mean_c, u_c)
                r_c = statp.tile([P, F2], f32, tag="r", name=f"r_{cc}")
                nc.vector.scalar_tensor_tensor(r_c, v_c, -2.0, sqc, Alu.mult, Alu.add)
                cumr = statp.tile([P, F2], f32, tag="cumr", name=f"cumr_{cc}")
                nc.vector.tensor_tensor_scan(
                    cumr, ones_scan, r_c, 0.0, Alu.mult, Alu.add
                )
                carry2 = carryb[:, 1:2]
                nc.tensor.matmul(
                    carry2,
                    lstrict_sb,
                    cumr[:, F2 - 1 : F2],
                    start=True,
                    stop=(cc == 0),
                )
                if cc > 0:
                    nc.tensor.matmul(
                        carry2,
                        ones_row,
                        g_prev[:, 1:2],
                        start=False,
                        stop=True,
                        skip_group_check=True,
                    )
                nc.tensor.matmul(
                    g_ps[:, 1:2],
                    ones_col,
                    cumr[:, F2 - 1 : F2],
                    start=(cc == 0),
                    stop=(cc == NCC - 1),
                    skip_group_check=True,
                )
                carry2_sb = statp.tile([P, 1], f32, tag="cs2", name=f"cs2_{cc}")
                nc.vector.tensor_copy(carry2_sb, carry2)
                var_c = statp.tile([P, F2], f32, tag="var", name=f"var_{cc}")
                nc.vector.scalar_tensor_tensor(
                    var_c, cumr, carry2_sb, rc, Alu.add, Alu.mult
                )
                std_c = statp.tile([P, F2], f32, tag="std", name=f"std_{cc}")
                nc.scalar.activation(std_c, var_c, Act.Sqrt, bias=eps_sb)
                inv_c = statp.tile([P, F2], f32, tag="inv", name=f"inv_{cc}")
                nc.vector.reciprocal(inv_c, std_c)
                nminv_c = statp.tile([P, F2], f32, tag="nminv", name=f"nm_{cc}")
                nc.vector.scalar_tensor_tensor(
                    nminv_c, mean_c, -1.0, inv_c, Alu.mult, Alu.mult
                )
                # ---- normalize (fully in place in the x tiles) ----
                # reshape compact stats into [1, HB] rows (SBUF->SBUF DMA),
                # then replicate across partitions on the idle GPSIMD engine
                PPH = HB // F2
                for h in range(CC // HB):
                    irow = rowp.tile([1, HB], f32, tag="brow", name=f"ir_{cc}_{h}")
                    nc.sync.dma_start(irow, inv_c[h * PPH : (h + 1) * PPH, :])
                    nrow = rowp.tile([1, HB], f32, tag="brow", name=f"nr_{cc}_{h}")
                    nc.sync.dma_start(nrow, nminv_c[h * PPH : (h + 1) * PPH, :])
                    bci = bcp.tile([P, HB], f32, tag="bc", name=f"bci_{cc}_{h}")
                    nc.gpsimd.partition_broadcast(bci, irow)
                    bcm = bcp.tile([P, HB], f32, tag="bc", name=f"bcm_{cc}_{h}")
                    nc.gpsimd.partition_broadcast(bcm, nrow)
                    for j in range(NCH):
                        sl = xts[j][:, h * HB : (h + 1) * HB]
                        nc.vector.tensor_mul(sl, sl, bci)
                        nc.vector.tensor_add(sl, sl, bcm)
                for j in range(NCH):
                    nc.scalar.activation(
                        xts[j],
                        xts[j],
                        Act.Identity,
                        bias=beta_sb[:, j : j + 1],
                        scale=gamma_sb[:, j : j + 1],
                    )
                    nc.sync.dma_start(
                        out[j * P : (j + 1) * P, t0 : t0 + CC], xts[j]
                    )

    nc.finalize()
    return nc


def _make_consts():
    t = np.arange(T, dtype=np.float64).reshape(NCC, P, F2).transpose(1, 0, 2)
    recip5 = np.ascontiguousarray((1.0 / (C * (t + 1.0))).astype(np.float32))
    lstrict = np.triu(np.ones((P, P), dtype=np.float32), k=1)
    return lstrict, recip5


def kernel(x, gamma, beta):
    global _PROG
    from concourse import bass_utils

    x = np.ascontiguousarray(np.asarray(x, dtype=np.float32))
    gamma = np.asarray(gamma, dtype=np.float32).reshape(C)
    beta = np.asarray(beta, dtype=np.float32).reshape(C)

    if _PROG is None:
        _PROG = _build_program()

    lstrict, recip5 = _make_consts()
    gamma_pc = np.ascontiguousarray(gamma.reshape(NCH, P).T)
    beta_pc = np.ascontiguousarray(beta.reshape(NCH, P).T)

    in_maps = [
        {
            "x": np.ascontiguousarray(x[b]),
            "lstrict": lstrict,
            "recip5": recip5,
            "gamma_pc": gamma_pc,
            "beta_pc": beta_pc,
        }
        for b in range(B)
    ]
    res = bass_utils.run_bass_kernel_spmd(_PROG, in_maps, core_ids=list(range(B)))
    return np.stack([res.results[b]["out"] for b in range(B)], axis=0)


# revision 39
# speedup vs baseline: 1.0936x; 1.0936x over previous
"""Cumulative LayerNorm (cLN) Trainium2 Bass kernel.

x: [B=8, C=512, T=16000] fp32.  Per (b, t):
    mean[t] = cumsum_t(sum_c x) / (C*(t+1))
    var[t]  = cumsum_t(sum_c (x - mean[t'])^2) / (C*(t+1))
    out     = (x - mean) / sqrt(var + eps) * gamma + beta

Expansion used on-device (exact in real arithmetic):
    sum_c (x[c,t'] - mean[t'])^2 = ssq[t'] - 2*mean[t']*s1[t'] + C*mean[t']^2

Sharding: data-parallel over batch, one batch per NeuronCore (8 cores).

Per-core pipeline, T processed in 5 chunks of 3200 so x is read from HBM only
once (the chunk stays resident in SBUF between the stats pass and the
normalization pass):
  Stats:   reduce over C via PE matmuls with an all-ones [128,1] stationary
           operand into PSUM rows s1/ssq [1,400];
           squares on ACT; rows evacuated to SBUF and DMA-reshaped into the
           chunk's compact scan layout [128p, 25f] (t_local = p*25 + f).
  Scan:    per-partition prefix sums via DVE tensor_tensor_scan; cross-
           partition carry via a strict-lower-triangular PE matmul; cross-
           chunk carry via a PSUM-accumulated grand total (g) broadcast with a
           second accumulating matmul; pointwise stats; inv = 1/sqrt(var+eps)
           (ACT sqrt + DVE reciprocal); nminv = -mean*inv.
  Norm:    inv/nminv reshaped to [1, 1600] rows (SBUF->SBUF DMA) and
           replicated across all 128 partitions by GPSIMD partition_broadcast
           (no HBM traffic); normalization runs fully in place in the x tiles
           (DVE mul + add, then one ACT affine folding gamma/beta); DMA out.

The ssq reduction matmuls use float32r (full-rate fp32): their input is the
ACT square with a float32r-rounded output, which the BIR verifier requires.
The s1 matmuls consume raw DMA-loaded x and must stay plain fp32 (4 cyc/row).

Built with Bacc (not raw Bass): its compile() pass legalizes multi-wait
instructions into event-semaphore chains — TRN2 hardware instructions can
carry only ONE sync wait.
"""

import numpy as np

B, C, T = 8, 512, 16000
P = 128
NCH = C // P        # 4 chunks of channels
CC = 3200           # t-chunk (must be P * F2 and divide T)
NCC = T // CC       # 5
F2 = CC // P        # 25: compact scan layout free dim per chunk
KB = 400            # PSUM-row block (<=512 fp32, 400 = 16*25)
NKB = CC // KB      # 8 blocks per chunk
HB = 1600           # normalization half-chunk
EPS = 1e-8

_PROG = None


def _build_program():
    from contextlib import ExitStack

    import concourse.bass as bass
    import concourse.tile as tile
    from concourse import bacc, mybir

    f32 = mybir.dt.float32
    f32r = mybir.dt.float32r
    Alu = mybir.AluOpType
    Act = mybir.ActivationFunctionType

    nc = bacc.Bacc("TRN2", debug=False)
    x = nc.dram_tensor("x", [C, T], f32, kind="ExternalInput").ap()
    lstrict = nc.dram_tensor("lstrict", [P, P], f32, kind="ExternalInput").ap()
    recip5 = nc.dram_tensor("recip5", [P, NCC, F2], f32, kind="ExternalInput").ap()
    gamma_pc = nc.dram_tensor("gamma_pc", [P, NCH], f32, kind="ExternalInput").ap()
    beta_pc = nc.dram_tensor("beta_pc", [P, NCH], f32, kind="ExternalInput").ap()
    out = nc.dram_tensor("out", [C, T], f32, kind="ExternalOutput").ap()

    with tile.TileContext(nc) as tc:
        with ExitStack() as ctx:
            singles = ctx.enter_context(tc.tile_pool(name="singles", bufs=1))
            xtp = ctx.enter_context(tc.tile_pool(name="xtp", bufs=11))
            sqp_pool = ctx.enter_context(tc.tile_pool(name="sqp_pool", bufs=4))
            bcp = ctx.enter_context(tc.tile_pool(name="bcp", bufs=3))
            rowp = ctx.enter_context(tc.tile_pool(name="rowp", bufs=4))
            statp = ctx.enter_context(tc.tile_pool(name="statp", bufs=2))
            ps_stat = ctx.enter_context(
                tc.tile_pool(name="ps_stat", bufs=6, space="PSUM")
            )
            ps_carry = ctx.enter_context(
                tc.tile_pool(name="ps_carry", bufs=1, space="PSUM")
            )
            ps_g = ctx.enter_context(tc.tile_pool(name="ps_g", bufs=1, space="PSUM"))

            # ---- constants ----
            ones_col = singles.tile([P, 1], f32)
            nc.vector.memset(ones_col, 1.0)
            ones_row = singles.tile([1, P], f32)
            nc.vector.memset(ones_row, 1.0)
            ones_scan = singles.tile([P, F2], f32)
            nc.vector.memset(ones_scan, 1.0)
            lstrict_sb = singles.tile([P, P], f32)
            nc.sync.dma_start(lstrict_sb, lstrict)
            recip_sb = singles.tile([P, NCC, F2], f32)
            nc.sync.dma_start(recip_sb, recip5)
            gamma_sb = singles.tile([P, NCH], f32)
            nc.sync.dma_start(gamma_sb, gamma_pc)
            beta_sb = singles.tile([P, NCH], f32)
            nc.sync.dma_start(beta_sb, beta_pc)
            eps_sb = singles.tile([P, 1], f32)
            nc.vector.memset(eps_sb, EPS)

            # grand totals over processed chunks: col 0 = sum(s1), col 1 = sum(r)
            g_ps = ps_g.tile([1, 2], f32, tag="g")

            for cc in range(NCC):
                t0 = cc * CC
                # ---- load chunk ----
                xts = []
                for j in range(NCH):
                    xtr = xtp.tile([P, CC], f32r, tag="xt", name=f"xt_{cc}_{j}")
                    nc.sync.dma_start(
                        xtr.bitcast(f32), x[j * P : (j + 1) * P, t0 : t0 + CC]
                    )
                    xts.append(xtr.bitcast(f32))

                # ---- stats: channel reductions ----
                s1c = statp.tile([P, F2], f32, tag="s1c", name=f"s1c_{cc}")
                sqc = statp.tile([P, F2], f32, tag="sqc", name=f"sqc_{cc}")
                for kp in range(NKB // 2):
                    xsqs = []
                    for j in range(NCH):
                        xsq = sqp_pool.tile(
                            [P, 2 * KB], f32r, tag="xsq", name=f"xsq_{cc}_{kp}_{j}"
                        )
                        nc.scalar.square(
                            xsq, xts[j][:, kp * 2 * KB : (kp + 1) * 2 * KB]
                        )
                        xsqs.append(xsq)
                    for k2 in range(2):
                        k = kp * 2 + k2
                        s1p = ps_stat.tile([1, KB], f32, tag="st", name=f"s1p_{cc}_{k}")
                        sqp = ps_stat.tile([1, KB], f32, tag="st", name=f"sqp_{cc}_{k}")
                        for j in range(NCH):
                            nc.tensor.matmul(
                                s1p,
                                ones_col,
                                xts[j][:, k * KB : (k + 1) * KB],
                                start=(j == 0),
                                stop=(j == NCH - 1),
                            )
                        for j in range(NCH):
                            nc.tensor.matmul(
                                sqp,
                                ones_col.bitcast(f32r),
                                xsqs[j][:, k2 * KB : (k2 + 1) * KB],
                                start=(j == 0),
                                stop=(j == NCH - 1),
                            )
                        s1row = rowp.tile(
                            [1, KB], f32, tag="rows", name=f"s1r_{cc}_{k}"
                        )
                        nc.vector.tensor_copy(s1row, s1p)
                        sqrow = rowp.tile(
                            [1, KB], f32, tag="rows", name=f"sqr_{cc}_{k}"
                        )
                        nc.scalar.copy(sqrow, sqp)
                        # 400 t's = 16 partitions x 25 in the chunk scan layout
                        nc.sync.dma_start(s1c[16 * k : 16 * k + 16, :], s1row)
                        nc.sync.dma_start(sqc[16 * k : 16 * k + 16, :], sqrow)

                # ---- scan + pointwise stats (compact [128, 25]) ----
                if cc > 0:
                    g_prev = statp.tile([1, 2], f32, tag="gprev", name=f"gp_{cc}")
                    nc.vector.tensor_copy(g_prev, g_ps)
                cum1 = statp.tile([P, F2], f32, tag="cum1", name=f"cum1_{cc}")
                nc.vector.tensor_tensor_scan(
                    cum1, ones_scan, s1c, 0.0, Alu.mult, Alu.add
                )
                carryb = ps_carry.tile([P, 2], f32, tag="c", name=f"c_{cc}")
                carry1 = carryb[:, 0:1]
                nc.tensor.matmul(
                    carry1,
                    lstrict_sb,
                    cum1[:, F2 - 1 : F2],
                    start=True,
                    stop=(cc == 0),
                )
                if cc > 0:
                    nc.tensor.matmul(
                        carry1,
                        ones_row,
                        g_prev[:, 0:1],
                        start=False,
                        stop=True,
                        skip_group_check=True,
                    )
                nc.tensor.matmul(
                    g_ps[:, 0:1],
                    ones_col,
                    cum1[:, F2 - 1 : F2],
                    start=(cc == 0),
                    stop=(cc == NCC - 1),
                    skip_group_check=True,
                )
                carry1_sb = statp.tile([P, 1], f32, tag="cs1", name=f"cs1_{cc}")
                nc.vector.tensor_copy(carry1_sb, carry1)
                rc = recip_sb[:, cc, :]
                mean_c = statp.tile([P, F2], f32, tag="mean", name=f"mean_{cc}")
                nc.vector.scalar_tensor_tensor(
                    mean_c, cum1, carry1_sb, rc, Alu.add, Alu.mult
                )
                u_c = statp.tile([P, F2], f32, tag="u", name=f"u_{cc}")
                nc.vector.scalar_tensor_tensor(
                    u_c, mean_c, -float(C) / 2.0, s1c, Alu.mult, Alu.add
                )
                v_c = statp.tile([P, F2], f32, tag="v", name=f"v_{cc}")
                nc.vector.tensor_mul(v_c, mean_c, u_c)
                r_c = statp.tile([P, F2], f32, tag="r", name=f"r_{cc}")
                nc.vector.scalar_tensor_tensor(r_c, v_c, -2.0, sqc, Alu.mult, Alu.add)
                cumr = statp.tile([P, F2], f32, tag="cumr", name=f"cumr_{cc}")
                nc.vector.tensor_tensor_scan(
                    cumr, ones_scan, r_c, 0.0, Alu.mult, Alu.add
                )
                carry2 = carryb[:, 1:2]
                nc.tensor.matmul(
                    carry2,
                    lstrict_sb,
                    cumr[:, F2 - 1 : F2],
                    start=True,
                    stop=(cc == 0),
                )
                if cc > 0:
                    nc.tensor.matmul(
                        carry2,
                        ones_row,
                        g_prev[:, 1:2],
                        start=False,
                        stop=True,
                        skip_group_check=True,
                    )
                nc.tensor.matmul(
                    g_ps[:, 1:2],
                    ones_col,
                    cumr[:, F2 - 1 : F2],
                    start=(cc == 0),
                    stop=(cc == NCC - 1),
                    skip_group_check=True,
                )
                carry2_sb = statp.tile([P, 1], f32, tag="cs2", name=f"cs2_{cc}")
                nc.vector.tensor_copy(carry2_sb, carry2)
                var_c = statp.tile([P, F2], f32, tag="var", name=f"var_{cc}")
                nc.vector.scalar_tensor_tensor(
                    var_c, cumr, carry2_sb, rc, Alu.add, Alu.mult
                )
                std_c = statp.tile([P, F2], f32, tag="std", name=f"std_{cc}")
                nc.scalar.activation(std_c, var_c, Act.Sqrt, bias=eps_sb)
                inv_c = statp.tile([P, F2], f32, tag="inv", name=f"inv_{cc}")
                nc.vector.reciprocal(inv_c, std_c)
                nminv_c = statp.tile([P, F2], f32, tag="nminv", name=f"nm_{cc}")
                nc.vector.scalar_tensor_tensor(
                    nminv_c, mean_c, -1.0, inv_c, Alu.mult, Alu.mult
                )
                # ---- normalize (fully in place in the x tiles) ----
                # reshape compact stats into [1, HB] rows (SBUF->SBUF DMA),
                # then replicate across partitions on the idle GPSIMD engine
                PPH = HB // F2
                for h in range(CC // HB):
                    irow = rowp.tile([1, HB], f32, tag="brow", name=f"ir_{cc}_{h}")
                    nc.sync.dma_start(irow, inv_c[h * PPH : (h + 1) * PPH, :])
                    nrow = rowp.tile([1, HB], f32, tag="brow", name=f"nr_{cc}_{h}")
                    nc.sync.dma_start(nrow, nminv_c[h * PPH : (h + 1) * PPH, :])
                    bci = bcp.tile([P, HB], f32, tag="bc", name=f"bci_{cc}_{h}")
                    nc.gpsimd.partition_broadcast(bci, irow)
                    bcm = bcp.tile([P, HB], f32, tag="bc", name=f"bcm_{cc}_{h}")
                    nc.gpsimd.partition_broadcast(bcm, nrow)
                    for j in range(NCH):
                        sl = xts[j][:, h * HB : (h + 1) * HB]
                        nc.vector.tensor_mul(sl, sl, bci)
                        nc.vector.tensor_add(sl, sl, bcm)
                        # per-half affine + store: the first half streams out
                        # while the second half is still multiplying
                        nc.scalar.activation(
                            sl,
                            sl,
                            Act.Identity,
                            bias=beta_sb[:, j : j + 1],
                            scale=gamma_sb[:, j : j + 1],
                        )
                        nc.sync.dma_start(
                            out[j * P : (j + 1) * P, t0 + h * HB : t0 + (h + 1) * HB],
                            sl,
                        )

    nc.finalize()
    return nc


def _make_consts():
    t = np.arange(T, dtype=np.float64).reshape(NCC, P, F2).transpose(1, 0, 2)
    recip5 = np.ascontiguousarray((1.0 / (C * (t + 1.0))).astype(np.float32))
    lstrict = np.triu(np.ones((P, P), dtype=np.float32), k=1)
    return lstrict, recip5


def kernel(x, gamma, beta):
    global _PROG
    from concourse import bass_utils

    x = np.ascontiguousarray(np.asarray(x, dtype=np.float32))
    gamma = np.asarray(gamma, dtype=np.float32).reshape(C)
    beta = np.asarray(beta, dtype=np.float32).reshape(C)

    if _PROG is None:
        _PROG = _build_program()

    lstrict, recip5 = _make_consts()
    gamma_pc = np.ascontiguousarray(gamma.reshape(NCH, P).T)
    beta_pc = np.ascontiguousarray(beta.reshape(NCH, P).T)

    in_maps = [
        {
            "x": np.ascontiguousarray(x[b]),
            "lstrict": lstrict,
            "recip5": recip5,
            "gamma_pc": gamma_pc,
            "beta_pc": beta_pc,
        }
        for b in range(B)
    ]
    res = bass_utils.run_bass_kernel_spmd(_PROG, in_maps, core_ids=list(range(B)))
    return np.stack([res.results[b]["out"] for b in range(B)], axis=0)
